# revision 24
# baseline (speedup 1.0000x reference)
"""Causal multi-head attention on 8 Trainium2 NeuronCores.

Sharding: data-parallel over batch (B=2) x tensor-parallel over heads
(16 heads -> 4 groups of 4). Core c handles batch c//4, head group c%4.
Each core computes q/k/v projections for its 4 heads, causal flash
attention, and a partial output projection (row slice of Wo); the host
sums the 4 partials per batch element.

All operands arrive pre-transposed from the host (xT[d,s], wT[d,e],
woT[e,o]) so the PE never runs transpose-mode ops. Matmuls run in bf16
(fp32 PSUM accumulation). The softmax row-sum is fused into the
o^T = [v|1]^T P^T matmul via an appended ones column; normalization
(broadcast rowsum via K=1 matmul reading partition 64, fast-approx
reciprocal, divide) stays in fp32.

Scheduling: everything is software-pipelined through the QK/exp slot
stream. Only the first q/k projection chunk runs inline; all later
projection chunks, V projections, AV matmuls, softmax epilogues, and
output projections drain from two work queues (projection-side and
attention-side) between the QK slots, with marker deadlines forcing
projection chunks to finish just before the attention group that reads
them. This keeps the PE fed while ACT runs the softmax exps from ~5us
onward instead of idling through a separate projection phase. The final
(q-chunk, head-pair) group pipelines its AV matmuls inline behind its
QK/exp slots and runs its epilogue + output projection immediately,
shortening the serial tail.
"""

import numpy as np
import ml_dtypes

import concourse.bacc as bacc
import concourse.bass as bass
import concourse.tile as tile
from concourse import bass_utils, mybir

B, S, D, H = 2, 2048, 1024, 16
DK = 64
NH = 4                 # heads per core
E = NH * DK            # 256: per-core head-dim slice
SCALE = 1.0 / 8.0      # 1/sqrt(DK)

F32 = mybir.dt.float32
F32R = mybir.dt.float32r
BF16 = mybir.dt.bfloat16

QC = 512               # q-chunk (columns per attention tile)
NQC = S // QC          # 4
NKB = S // 128         # 16 k-blocks


def _emit(tc, nc, xT_d, wqT_d, wkT_d, wvT_d, woT_d, yT_d, mask_d, ones_d):
    const = tc.alloc_tile_pool(name="const", bufs=1)
    perm = tc.alloc_tile_pool(name="perm", bufs=1)
    p01 = tc.alloc_tile_pool(name="p01", bufs=1)
    work = tc.alloc_tile_pool(name="work", bufs=3)
    small = tc.alloc_tile_pool(name="small", bufs=2)

    mask = const.tile([128, 128], BF16)
    ones_f32 = const.tile([128, 64], F32)
    ones128 = const.tile([128, 64], F32R)

    woT = perm.tile([128, 2, D], BF16)   # woT[p, ec, o] = wo[o, ec*128+p]
    qT = perm.tile([128, 2, S], BF16)    # qT[p, ec, s] = q[s, ec*128+p]
    kT = perm.tile([128, 2, S], BF16)
    v_sb = perm.tile([128, NKB, NH, DK + 1], BF16)  # [.., 64] = ones column

    xT = p01.tile([128, 8, S], BF16)     # xT[p, dc, s] = x[s, dc*128+p]
    wqT = p01.tile([128, 8, E], BF16)    # wqT[p, dc, e] = wq[e, dc*128+p]
    wkT = p01.tile([128, 8, E], BF16)
    wvT = p01.tile([128, 8, E], BF16)

    # Coalesced DMAs (one 3D-AP transfer per tensor/chunk): each dma_start
    # costs ~600ns of issue time on its queue engine, so fewer+bigger wins.
    # Queues are spread so the first projection inputs land by ~2us.
    # xT_d is host-preblocked: row-major [sc, dc, p, t] so every (sc, dc)
    # 128x512 chunk is a contiguous 128KB slab (strided DRAM reads run ~4x
    # slower per DMA engine)
    def x_src(sc, dlo, dhi):
        return bass.AP(
            tensor=xT_d.tensor,
            offset=xT_d.offset + (sc * 8 + dlo) * 128 * QC,
            ap=[[QC, 128], [128 * QC, dhi - dlo], [1, QC]],
        )

    def w_src(w_d, dlo, dhi):
        return bass.AP(
            tensor=w_d.tensor, offset=w_d.offset + dlo * 128 * E,
            ap=[[E, 128], [128 * E, dhi - dlo], [1, E]],
        )

    def xd(eng, sc, dlo, dhi):
        eng.dma_start(out=xT[:, dlo:dhi, sc * QC:(sc + 1) * QC],
                      in_=x_src(sc, dlo, dhi))

    def wd(eng, w_d, wT, dlo, dhi):
        eng.dma_start(out=wT[:, dlo:dhi, :], in_=w_src(w_d, dlo, dhi))

    # one ~128KB transfer per dma_start, fanned out over many DMA engines;
    # per-queue issue order follows compute's need order
    xd(nc.sync, 0, 0, 1)
    wd(nc.scalar, wqT_d, wqT, 0, 2)
    wd(nc.gpsimd, wkT_d, wkT, 0, 2)
    xd(nc.sync, 0, 1, 2)
    wd(nc.scalar, wqT_d, wqT, 2, 4)
    wd(nc.gpsimd, wkT_d, wkT, 2, 4)
    xd(nc.sync, 0, 4, 6)
    wd(nc.scalar, wqT_d, wqT, 4, 8)
    wd(nc.gpsimd, wkT_d, wkT, 4, 8)
    xd(nc.gpsimd, 0, 2, 4)
    xd(nc.gpsimd, 0, 6, 8)
    nc.scalar.dma_start(out=mask, in_=mask_d)
    nc.scalar.dma_start(out=ones_f32, in_=ones_d)
    xd(nc.sync, 1, 0, 2)
    wd(nc.scalar, wvT_d, wvT, 0, 4)
    xd(nc.gpsimd, 1, 2, 4)
    xd(nc.sync, 1, 4, 6)
    wd(nc.scalar, wvT_d, wvT, 4, 8)
    xd(nc.gpsimd, 1, 6, 8)
    xd(nc.sync, 2, 0, 2)
    xd(nc.scalar, 2, 2, 4)
    xd(nc.gpsimd, 3, 0, 2)
    xd(nc.sync, 2, 4, 6)
    xd(nc.scalar, 2, 6, 8)
    xd(nc.gpsimd, 3, 4, 6)
    xd(nc.sync, 3, 6, 8)
    xd(nc.scalar, 3, 2, 4)
    for ec in range(2):
        nc.gpsimd.dma_start(
            out=woT[:, ec, :],
            in_=bass.AP(tensor=woT_d.tensor, offset=woT_d.offset + ec * 128 * D,
                        ap=[[D, 128], [1, D]]),
        )
    # ones row for the rowsum broadcast (row 64 used as lhsT)
    nc.vector.tensor_copy(ones128, ones_f32)

    # ones column of v_sb (written once; strided 3D AP)
    ones_ap = bass.AP(
        tensor=v_sb.tensor, offset=v_sb.offset + DK,
        ap=[v_sb.ap[0], [NH * (DK + 1), NKB], [DK + 1, NH]],
    )
    src64 = bass.AP(
        tensor=ones_f32.tensor, offset=ones_f32.offset,
        ap=[ones_f32.ap[0], [4, NKB], [1, NH]],
    )
    nc.vector.tensor_copy(ones_ap, src64)

    with tc.tile_pool(name="psS", bufs=1, space="PSUM") as ps_S, \
         tc.tile_pool(name="psO", bufs=1, space="PSUM") as ps_o, \
         tc.tile_pool(name="psY", bufs=1, space="PSUM") as ps_y:

        ncopy = [0]

        def copy(dst, src):
            # DVE-heavy split: ACT runs the softmax exps, DVE has headroom
            if ncopy[0] % 4 != 3:
                nc.vector.tensor_copy(dst, src)
            else:
                nc.scalar.copy(dst, src)
            ncopy[0] += 1

        # ---- background unit builders ----
        def proj_unit(w_t, outT, ec, sc):
            def u():
                ps = ps_y.tile([128, QC], F32, tag="y", bufs=2, name="psp")
                for dc in range(8):
                    nc.tensor.matmul(
                        ps,
                        lhsT=w_t[:, dc, ec * 128:(ec + 1) * 128],
                        rhs=xT[:, dc, sc * QC:(sc + 1) * QC],
                        start=(dc == 0),
                        stop=(dc == 7),
                    )
                copy(outT[:, ec, sc * QC:(sc + 1) * QC], ps)
            return u

        def vproj_unit(sblk):
            def u():
                ps = ps_y.tile([128, E], F32, tag="y", bufs=2, name="psv")
                for dc in range(8):
                    nc.tensor.matmul(
                        ps,
                        lhsT=xT[:, dc, sblk * 128:(sblk + 1) * 128],
                        rhs=wvT[:, dc, :],
                        start=(dc == 0),
                        stop=(dc == 7),
                    )
                # scatter 4 heads into [.., l, 0:64]
                sap = bass.AP(
                    tensor=ps.tensor, offset=ps.offset,
                    ap=[ps.ap[0], [DK, NH], [1, DK]],
                )
                nc.vector.tensor_copy(v_sb[:, sblk, :, 0:DK], sap)
            return u

        def make_av(po_box, pts, kb, hp, kmax):
            def av():
                if po_box[0] is None:
                    po_box[0] = (
                        ps_o.tile([DK + 1, QC], F32, tag="o", bufs=2, name="poA"),
                        ps_o.tile([DK + 1, QC], F32, tag="o", bufs=2, name="poB"),
                    )
                poA, poB = po_box[0]
                pT, cs = pts[kb]
                for hi, po in ((0, poA), (1, poB)):
                    nc.tensor.matmul(
                        po[:, cs:QC],
                        lhsT=v_sb[:, kb, 2 * hp + hi, :],
                        rhs=pT[:, hi, cs:QC],
                        start=(kb == 0),
                        stop=(kb == kmax - 1),
                    )
            return av

        def make_epilogue(po_box, oT, hp, c0=0, c1=QC):
            def epi():
                poA, poB = po_box[0]
                w = c1 - c0
                oA_sb = small.tile([DK + 1, w], F32R, tag="osb", bufs=4)
                oB_sb = small.tile([DK + 1, w], F32R, tag="osb", bufs=4)
                nc.vector.tensor_copy(oA_sb, poA[:, c0:c1])
                nc.vector.tensor_copy(oB_sb, poB[:, c0:c1])
                for hi, o_sb in ((0, oA_sb), (1, oB_sb)):
                    # broadcast rowsum (row 64) to 64 partitions via K=1
                    # matmul reading partition 64 (row group 64)
                    ps_bc = ps_y.tile([64, w], F32, tag="y", bufs=2, name="psbc")
                    nc.tensor.matmul(
                        ps_bc,
                        lhsT=ones128[DK:DK + 1, :],
                        rhs=o_sb[DK:DK + 1, :],
                        start=True,
                        stop=True,
                    )
                    rec = small.tile([64, w], F32, tag="rec", bufs=2)
                    nc.vector.reciprocal_approx_fast(rec, ps_bc)
                    if hi == 0:
                        nc.vector.tensor_mul(oT[0:DK, hp, c0:c1], o_sb[0:DK, :], rec)
                    else:
                        tmpB = small.tile([64, w], BF16, tag="tmpB", bufs=2)
                        nc.vector.tensor_mul(tmpB, o_sb[0:DK, :], rec)
                        # partition shift 0-63 -> 64-127 via sbuf->sbuf DMA
                        # (scalar queue: idle mid-run, keeps it off the y path)
                        nc.scalar.dma_start(out=oT[DK:128, hp, c0:c1], in_=tmpB)
            return epi

        def make_out_proj(qc, oT, c0=0, c1=QC):
            units = []
            for dc in range(8):
                def u(dc=dc, qc=qc, oT=oT):
                    w = c1 - c0
                    psy = ps_y.tile([128, w], F32, tag="y", bufs=2, name="psy")
                    for ec in range(2):
                        nc.tensor.matmul(
                            psy,
                            lhsT=woT[:, ec, dc * 128:(dc + 1) * 128],
                            rhs=oT[:, ec, c0:c1],
                            start=(ec == 0),
                            stop=(ec == 1),
                        )
                    y_sb = work.tile([128, w], BF16, tag="ysb", bufs=3)
                    copy(y_sb, psy)
                    nc.sync.dma_start(
                        out=yT_d[dc * 128:(dc + 1) * 128,
                                 qc * QC + c0:qc * QC + c1],
                        in_=y_sb,
                    )
                units.append(u)
            return units

        # ---- work queues ----
        # projq: (weight, key, fn) -- FIFO, marker deadlines force chunks
        # to complete before the attention group that reads them.
        # attq: (weight, vp_need, fn) -- FIFO; av units record the highest
        # vproj block they read so pops can force those vprojs first.
        projq = []
        attq = []
        proj_drained = [0]
        att_drained = [0]
        pT_made = [0]
        pT_used = [0]

        projq.append((8, ("q", 0, 1), proj_unit(wqT, qT, 1, 0)))
        projq.append((8, ("k", 0, 1), proj_unit(wkT, kT, 1, 0)))
        for s in range(4):
            projq.append((4, ("vp", s), vproj_unit(s)))
        for sc in (1, 2, 3):
            for ec in (0, 1):
                projq.append((8, ("q", sc, ec), proj_unit(wqT, qT, ec, sc)))
                projq.append((8, ("k", sc, ec), proj_unit(wkT, kT, ec, sc)))
            for s in range(4 * sc, 4 * sc + 4):
                projq.append((4, ("vp", s), vproj_unit(s)))

        def pop_proj():
            w, _, u = projq.pop(0)
            u()
            proj_drained[0] += w

        def drain_proj_through(key):
            while any(e[1] == key for e in projq):
                pop_proj()

        def pop_att():
            w, need, u = attq[0]
            if need is not None:
                while any(e[1][0] == "vp" and e[1][1] <= need for e in projq):
                    pop_proj()
            attq.pop(0)
            u()
            att_drained[0] += w
            if need is not None:
                pT_used[0] += 1

        # pacing totals over the 64 pre-last-group slots
        W_PROJ = sum(e[0] for e in projq)                 # 176
        W_ATT = (2 * (4 + 8 + 12 + 16) - 16) + 7 + 3 * 8 * 2   # 119
        slot_i = [0]

        # ---- inline start: first q/k chunk, then the slot stream ----
        proj_unit(wqT, qT, 0, 0)()
        proj_unit(wkT, kT, 0, 0)()

        for qc in range(NQC):
            oT = work.tile([128, 2, QC], BF16, tag="oT", bufs=2)
            kmax = 4 * (qc + 1)
            for hp in range(2):
                last = (qc == NQC - 1 and hp == 1)
                drain_proj_through(("q", qc, hp))
                pts = {}
                po_box = [None]
                avs = [make_av(po_box, pts, kb, hp, kmax) for kb in range(kmax)]
                pending = [0]
                for kb in range(kmax):
                    if kb % 4 == 0:
                        drain_proj_through(("k", kb // 4, hp))
                    # S^T = k q^T, 2-head row-tiled pair, causally narrowed
                    cs = max(0, kb * 128 - qc * QC)
                    psS = ps_S.tile([128, 2, QC], F32, tag="S", bufs=2)
                    for hi in range(2):
                        nc.tensor.matmul(
                            psS[:, hi, cs:QC],
                            lhsT=kT[hi * 64:(hi + 1) * 64, hp,
                                    kb * 128:(kb + 1) * 128],
                            rhs=qT[hi * 64:(hi + 1) * 64, hp,
                                   qc * QC + cs:(qc + 1) * QC],
                            start=True,
                            stop=True,
                        )
                    pT = work.tile([128, 2, QC], BF16, tag="pT", bufs=30)
                    pts[kb] = (pT, cs)
                    pT_made[0] += 1
                    nc.scalar.activation(
                        pT[:, :, cs:QC],
                        psS[:, :, cs:QC],
                        mybir.ActivationFunctionType.Exp,
                        scale=SCALE,
                    )
                    if kb >= 4 * qc:  # diagonal band: zero the upper triangle
                        mask2 = bass.AP(
                            tensor=mask.tensor, offset=mask.offset,
                            ap=[mask.ap[0], [0, 2], mask.ap[1]],
                        )
                        nc.vector.tensor_mul(
                            pT[:, :, cs:cs + 128],
                            pT[:, :, cs:cs + 128],
                            mask2,
                        )
                    if last:
                        # drain the leftover queues, then pipeline this
                        # group's AV matmuls inline behind the exps
                        b = 3
                        while (attq or projq) and b > 0:
                            if attq:
                                w = attq[0][0]
                                pop_att()
                            else:
                                w = projq[0][0]
                                pop_proj()
                            b -= w
                        if not attq and not projq:
                            while pending[0] < kb:
                                avs[pending[0]]()
                                pending[0] += 1
                    else:
                        slot_i[0] += 1
                        at = min(3, (W_ATT * max(0, slot_i[0] - 4)) // 60
                                 - att_drained[0])
                        while attq and at > 0:
                            at -= attq[0][0]
                            pop_att()
                        pt = min(6, (W_PROJ * slot_i[0]) // 56 - proj_drained[0])
                        while projq and pt > 0:
                            pt -= projq[0][0]
                            pop_proj()
                        # backpressure: keep the pT pool from wrapping
                        while attq and pT_made[0] - pT_used[0] > 24:
                            pop_att()
                if last:
                    while attq or projq:
                        if attq:
                            pop_att()
                        else:
                            pop_proj()
                    while pending[0] < kmax:
                        avs[pending[0]]()
                        pending[0] += 1
                    # column-split tail: output projection of the first half
                    # overlaps the second half's epilogue on DVE
                    for h in range(2):
                        c0, c1 = h * (QC // 2), (h + 1) * (QC // 2)
                        make_epilogue(po_box, oT, hp, c0, c1)()
                        for u in make_out_proj(qc, oT, c0, c1):
                            u()
                else:
                    attq.extend((1, 4 * qc + 3, u) for u in avs)
                    attq.append((1, None, make_epilogue(po_box, oT, hp)))
                    if hp == 1:
                        attq.extend((2, None, u) for u in make_out_proj(qc, oT))
        while attq or projq:
            if attq:
                pop_att()
            else:
                pop_proj()

    for p in [small, work, p01, perm, const]:
        p.release()


_CACHE = {}


def _build():
    if "nc" in _CACHE:
        return _CACHE["nc"]
    nc = bacc.Bacc("TRN2", target_bir_lowering=False, debug=False, num_devices=8)
    xT_d = nc.dram_tensor("xT", [NQC * 8 * 128, QC], BF16, kind="ExternalInput").ap()
    wqT_d = nc.dram_tensor("wqT", [D, E], BF16, kind="ExternalInput").ap()
    wkT_d = nc.dram_tensor("wkT", [D, E], BF16, kind="ExternalInput").ap()
    wvT_d = nc.dram_tensor("wvT", [D, E], BF16, kind="ExternalInput").ap()
    woT_d = nc.dram_tensor("woT", [E, D], BF16, kind="ExternalInput").ap()
    yT_d = nc.dram_tensor("yT", [D, S], BF16, kind="ExternalOutput").ap()
    mask_d = nc.dram_tensor("maskc", [128, 128], BF16, kind="ExternalInput").ap()
    ones_d = nc.dram_tensor("onesc", [128, 64], F32, kind="ExternalInput").ap()
    with tile.TileContext(nc) as tc:
        _emit(tc, nc, xT_d, wqT_d, wkT_d, wvT_d, woT_d, yT_d, mask_d, ones_d)
    nc.compile()
    _CACHE["nc"] = nc
    return nc


_r = np.arange(128)
_MASK = np.where(_r[:, None] <= _r[None, :], 1.0, 0.0).astype(ml_dtypes.bfloat16)
_ONES = np.ones((128, 64), dtype=np.float32)

LAST_RESULT = None


def kernel(x, wq, wk, wv, wo):
    global LAST_RESULT
    nc = _build()
    bf = ml_dtypes.bfloat16
    x = np.asarray(x, dtype=np.float32)
    wq = np.asarray(wq, dtype=np.float32)
    wk = np.asarray(wk, dtype=np.float32)
    wv = np.asarray(wv, dtype=np.float32)
    wo = np.asarray(wo, dtype=np.float32)

    # pre-blocked layout: [sc, dc, p, t] row-major so each (sc, dc) chunk
    # is a contiguous 128KB slab for the DMA engines
    xT = [
        np.ascontiguousarray(
            x[b].T.reshape(8, 128, NQC, QC).transpose(2, 0, 1, 3)
        ).astype(bf).reshape(NQC * 8 * 128, QC)
        for b in range(B)
    ]
    wqT, wkT, wvT, woT = [], [], [], []
    for g in range(4):
        rows = slice(g * E, (g + 1) * E)
        wqT.append(np.ascontiguousarray(wq[rows].T).astype(bf))
        wkT.append(np.ascontiguousarray(wk[rows].T).astype(bf))
        wvT.append(np.ascontiguousarray(wv[rows].T).astype(bf))
        woT.append(np.ascontiguousarray(wo[:, rows].T).astype(bf))

    in_maps = []
    for c in range(8):
        b, g = c // 4, c % 4
        in_maps.append({
            "xT": xT[b],
            "wqT": wqT[g],
            "wkT": wkT[g],
            "wvT": wvT[g],
            "woT": woT[g],
            "maskc": _MASK,
            "onesc": _ONES,
        })

    res = bass_utils.run_bass_kernel_spmd(nc, in_maps, core_ids=list(range(8)))
    LAST_RESULT = res

    y = np.empty((B, S, D), dtype=np.float32)
    for b in range(B):
        acc = res.results[4 * b]["yT"].astype(np.float32)
        for g in range(1, 4):
            acc += res.results[4 * b + g]["yT"].astype(np.float32)
        y[b] = acc.T
    return y


# revision 26
# speedup vs baseline: 1.0623x; 1.0623x over previous
"""Causal multi-head attention on 8 Trainium2 NeuronCores.

Sharding: data-parallel over batch (B=2) x tensor-parallel over heads
(16 heads -> 4 groups of 4). Core c handles batch c//4, head group c%4.
Each core computes q/k/v projections for its 4 heads, causal flash
attention, and a partial output projection (row slice of Wo); the host
sums the 4 partials per batch element.

All operands arrive pre-transposed from the host (xT[d,s], wT[d,e],
woT[e,o]) so the PE never runs transpose-mode ops. Matmuls run in bf16
(fp32 PSUM accumulation). The softmax row-sum is fused into the
o^T = [v|1]^T P^T matmul via an appended ones column; normalization
(broadcast rowsum via K=1 matmul reading partition 64, fast-approx
reciprocal, divide) stays in fp32.

Scheduling: everything is software-pipelined through the QK/exp slot
stream. Only the first q/k projection chunk runs inline; all later
projection chunks, V projections, AV matmuls, softmax epilogues, and
output projections drain from two work queues (projection-side and
attention-side) between the QK slots, with marker deadlines forcing
projection chunks to finish just before the attention group that reads
them. This keeps the PE fed while ACT runs the softmax exps from ~5us
onward instead of idling through a separate projection phase. The final
(q-chunk, head-pair) group pipelines its AV matmuls inline behind its
QK/exp slots and runs its epilogue + output projection immediately,
shortening the serial tail.
"""

import numpy as np
import ml_dtypes

import concourse.bacc as bacc
import concourse.bass as bass
import concourse.tile as tile
from concourse import bass_utils, mybir

B, S, D, H = 2, 2048, 1024, 16
DK = 64
NH = 4                 # heads per core
E = NH * DK            # 256: per-core head-dim slice
SCALE = 1.0 / 8.0      # 1/sqrt(DK)

F32 = mybir.dt.float32
F32R = mybir.dt.float32r
BF16 = mybir.dt.bfloat16

QC = 512               # q-chunk (columns per attention tile)
NQC = S // QC          # 4
NKB = S // 128         # 16 k-blocks


def _emit(tc, nc, xT_d, wqT_d, wkT_d, wvT_d, woT_d, yT_d, mask_d, ones_d):
    const = tc.alloc_tile_pool(name="const", bufs=1)
    perm = tc.alloc_tile_pool(name="perm", bufs=1)
    p01 = tc.alloc_tile_pool(name="p01", bufs=1)
    work = tc.alloc_tile_pool(name="work", bufs=3)
    small = tc.alloc_tile_pool(name="small", bufs=2)

    mask = const.tile([128, 128], BF16)
    ones_f32 = const.tile([128, 64], F32)
    ones128 = const.tile([128, 64], F32R)

    woT = perm.tile([128, 2, D], BF16)   # woT[p, ec, o] = wo[o, ec*128+p]
    qT = perm.tile([128, 2, S], BF16)    # qT[p, ec, s] = q[s, ec*128+p]
    kT = perm.tile([128, 2, S], BF16)
    v_sb = perm.tile([128, NKB, NH, DK + 1], BF16)  # [.., 64] = ones column

    xT = p01.tile([128, 8, S], BF16)     # xT[p, dc, s] = x[s, dc*128+p]
    wqT = p01.tile([128, 8, E], BF16)    # wqT[p, dc, e] = wq[e, dc*128+p]
    wkT = p01.tile([128, 8, E], BF16)
    wvT = p01.tile([128, 8, E], BF16)

    # Coalesced DMAs (one 3D-AP transfer per tensor/chunk): each dma_start
    # costs ~600ns of issue time on its queue engine, so fewer+bigger wins.
    # Queues are spread so the first projection inputs land by ~2us.
    # xT_d is host-preblocked: row-major [sc, dc, p, t] so every (sc, dc)
    # 128x512 chunk is a contiguous 128KB slab (strided DRAM reads run ~4x
    # slower per DMA engine)
    def x_src(sc, dlo, dhi):
        return bass.AP(
            tensor=xT_d.tensor,
            offset=xT_d.offset + (sc * 8 + dlo) * 128 * QC,
            ap=[[QC, 128], [128 * QC, dhi - dlo], [1, QC]],
        )

    def w_src(w_d, dlo, dhi):
        return bass.AP(
            tensor=w_d.tensor, offset=w_d.offset + dlo * 128 * E,
            ap=[[E, 128], [128 * E, dhi - dlo], [1, E]],
        )

    def xd(eng, sc, dlo, dhi):
        eng.dma_start(out=xT[:, dlo:dhi, sc * QC:(sc + 1) * QC],
                      in_=x_src(sc, dlo, dhi))

    def wd(eng, w_d, wT, dlo, dhi):
        eng.dma_start(out=wT[:, dlo:dhi, :], in_=w_src(w_d, dlo, dhi))

    # one ~128KB transfer per dma_start, fanned out over many DMA engines;
    # per-queue issue order follows compute's need order
    xd(nc.sync, 0, 0, 1)
    wd(nc.scalar, wqT_d, wqT, 0, 2)
    wd(nc.gpsimd, wkT_d, wkT, 0, 2)
    xd(nc.sync, 0, 1, 2)
    wd(nc.scalar, wqT_d, wqT, 2, 4)
    wd(nc.gpsimd, wkT_d, wkT, 2, 4)
    xd(nc.sync, 0, 4, 6)
    wd(nc.scalar, wqT_d, wqT, 4, 8)
    wd(nc.gpsimd, wkT_d, wkT, 4, 8)
    xd(nc.gpsimd, 0, 2, 4)
    xd(nc.gpsimd, 0, 6, 8)
    nc.scalar.dma_start(out=mask, in_=mask_d)
    nc.scalar.dma_start(out=ones_f32, in_=ones_d)
    xd(nc.sync, 1, 0, 2)
    wd(nc.scalar, wvT_d, wvT, 0, 4)
    xd(nc.gpsimd, 1, 2, 4)
    xd(nc.sync, 1, 4, 6)
    wd(nc.scalar, wvT_d, wvT, 4, 8)
    xd(nc.gpsimd, 1, 6, 8)
    xd(nc.sync, 2, 0, 2)
    xd(nc.scalar, 2, 2, 4)
    xd(nc.gpsimd, 3, 0, 2)
    xd(nc.sync, 2, 4, 6)
    xd(nc.scalar, 2, 6, 8)
    xd(nc.gpsimd, 3, 4, 6)
    xd(nc.sync, 3, 6, 8)
    xd(nc.scalar, 3, 2, 4)
    for ec in range(2):
        nc.gpsimd.dma_start(
            out=woT[:, ec, :],
            in_=bass.AP(tensor=woT_d.tensor, offset=woT_d.offset + ec * 128 * D,
                        ap=[[D, 128], [1, D]]),
        )
    # ones row for the rowsum broadcast (row 64 used as lhsT)
    nc.vector.tensor_copy(ones128, ones_f32)

    # ones column of v_sb (written once; strided 3D AP)
    ones_ap = bass.AP(
        tensor=v_sb.tensor, offset=v_sb.offset + DK,
        ap=[v_sb.ap[0], [NH * (DK + 1), NKB], [DK + 1, NH]],
    )
    src64 = bass.AP(
        tensor=ones_f32.tensor, offset=ones_f32.offset,
        ap=[ones_f32.ap[0], [4, NKB], [1, NH]],
    )
    nc.vector.tensor_copy(ones_ap, src64)

    with tc.tile_pool(name="psS", bufs=1, space="PSUM") as ps_S, \
         tc.tile_pool(name="psO", bufs=1, space="PSUM") as ps_o, \
         tc.tile_pool(name="psY", bufs=1, space="PSUM") as ps_y:

        ncopy = [0]

        def copy(dst, src):
            # DVE-heavy split: ACT runs the softmax exps, DVE has headroom
            if ncopy[0] % 4 != 3:
                nc.vector.tensor_copy(dst, src)
            else:
                nc.scalar.copy(dst, src)
            ncopy[0] += 1

        # ---- background unit builders ----
        def proj_unit(w_t, outT, ec, sc):
            def u():
                ps = ps_y.tile([128, QC], F32, tag="y", bufs=2, name="psp")
                for dc in range(8):
                    nc.tensor.matmul(
                        ps,
                        lhsT=w_t[:, dc, ec * 128:(ec + 1) * 128],
                        rhs=xT[:, dc, sc * QC:(sc + 1) * QC],
                        start=(dc == 0),
                        stop=(dc == 7),
                    )
                copy(outT[:, ec, sc * QC:(sc + 1) * QC], ps)
            return u

        def vproj_unit(sblk):
            def u():
                ps = ps_y.tile([128, E], F32, tag="y", bufs=2, name="psv")
                for dc in range(8):
                    nc.tensor.matmul(
                        ps,
                        lhsT=xT[:, dc, sblk * 128:(sblk + 1) * 128],
                        rhs=wvT[:, dc, :],
                        start=(dc == 0),
                        stop=(dc == 7),
                    )
                # scatter 4 heads into [.., l, 0:64]
                sap = bass.AP(
                    tensor=ps.tensor, offset=ps.offset,
                    ap=[ps.ap[0], [DK, NH], [1, DK]],
                )
                nc.vector.tensor_copy(v_sb[:, sblk, :, 0:DK], sap)
            return u

        def make_av(po_box, pts, kb, hp, kmax):
            def av():
                if po_box[0] is None:
                    po_box[0] = (
                        ps_o.tile([DK + 1, QC], F32, tag="o", bufs=2, name="poA"),
                        ps_o.tile([DK + 1, QC], F32, tag="o", bufs=2, name="poB"),
                    )
                poA, poB = po_box[0]
                pT, cs = pts[kb]
                for hi, po in ((0, poA), (1, poB)):
                    nc.tensor.matmul(
                        po[:, cs:QC],
                        lhsT=v_sb[:, kb, 2 * hp + hi, :],
                        rhs=pT[:, hi, cs:QC],
                        start=(kb == 0),
                        stop=(kb == kmax - 1),
                    )
            return av

        def make_epilogue(po_box, oT, hp, c0=0, c1=QC):
            def epi():
                poA, poB = po_box[0]
                w = c1 - c0
                oA_sb = small.tile([DK + 1, w], F32R, tag="osb", bufs=4)
                oB_sb = small.tile([DK + 1, w], F32R, tag="osb", bufs=4)
                nc.vector.tensor_copy(oA_sb, poA[:, c0:c1])
                nc.vector.tensor_copy(oB_sb, poB[:, c0:c1])
                for hi, o_sb in ((0, oA_sb), (1, oB_sb)):
                    # broadcast rowsum (row 64) to 64 partitions via K=1
                    # matmul reading partition 64 (row group 64)
                    ps_bc = ps_y.tile([64, w], F32, tag="y", bufs=2, name="psbc")
                    nc.tensor.matmul(
                        ps_bc,
                        lhsT=ones128[DK:DK + 1, :],
                        rhs=o_sb[DK:DK + 1, :],
                        start=True,
                        stop=True,
                    )
                    rec = small.tile([64, w], F32, tag="rec", bufs=2)
                    nc.vector.reciprocal_approx_fast(rec, ps_bc)
                    if hi == 0:
                        nc.vector.tensor_mul(oT[0:DK, hp, c0:c1], o_sb[0:DK, :], rec)
                    else:
                        tmpB = small.tile([64, w], BF16, tag="tmpB", bufs=2)
                        nc.vector.tensor_mul(tmpB, o_sb[0:DK, :], rec)
                        # partition shift 0-63 -> 64-127 via sbuf->sbuf DMA
                        nc.sync.dma_start(out=oT[DK:128, hp, c0:c1], in_=tmpB)
            return epi

        def make_out_proj(qc, oT, c0=0, c1=QC):
            units = []
            for dc in range(8):
                def u(dc=dc, qc=qc, oT=oT):
                    w = c1 - c0
                    psy = ps_y.tile([128, w], F32, tag="y", bufs=2, name="psy")
                    for ec in range(2):
                        nc.tensor.matmul(
                            psy,
                            lhsT=woT[:, ec, dc * 128:(dc + 1) * 128],
                            rhs=oT[:, ec, c0:c1],
                            start=(ec == 0),
                            stop=(ec == 1),
                        )
                    y_sb = work.tile([128, w], BF16, tag="ysb", bufs=3)
                    copy(y_sb, psy)
                    nc.sync.dma_start(
                        out=yT_d[dc * 128:(dc + 1) * 128,
                                 qc * QC + c0:qc * QC + c1],
                        in_=y_sb,
                    )
                units.append(u)
            return units

        # ---- work queues ----
        # projq: (weight, key, fn) -- FIFO, marker deadlines force chunks
        # to complete before the attention group that reads them.
        # attq: (weight, vp_need, fn) -- FIFO; av units record the highest
        # vproj block they read so pops can force those vprojs first.
        projq = []
        attq = []
        proj_drained = [0]
        att_drained = [0]
        pT_made = [0]
        pT_used = [0]

        projq.append((8, ("q", 0, 1), proj_unit(wqT, qT, 1, 0)))
        projq.append((8, ("k", 0, 1), proj_unit(wkT, kT, 1, 0)))
        for s in range(4):
            projq.append((4, ("vp", s), vproj_unit(s)))
        for sc in (1, 2, 3):
            for ec in (0, 1):
                projq.append((8, ("q", sc, ec), proj_unit(wqT, qT, ec, sc)))
                projq.append((8, ("k", sc, ec), proj_unit(wkT, kT, ec, sc)))
            for s in range(4 * sc, 4 * sc + 4):
                projq.append((4, ("vp", s), vproj_unit(s)))

        def pop_proj():
            w, _, u = projq.pop(0)
            u()
            proj_drained[0] += w

        def drain_proj_through(key):
            while any(e[1] == key for e in projq):
                pop_proj()

        def pop_att():
            w, need, u = attq[0]
            if need is not None:
                while any(e[1][0] == "vp" and e[1][1] <= need for e in projq):
                    pop_proj()
            attq.pop(0)
            u()
            att_drained[0] += w
            if need is not None:
                pT_used[0] += 1

        # pacing totals over the 64 pre-last-group slots
        W_PROJ = sum(e[0] for e in projq)                 # 176
        W_ATT = (2 * (4 + 8 + 12 + 16) - 16) + 7 + 3 * 8 * 2   # 119
        slot_i = [0]

        # ---- inline start: first q/k chunk, then the slot stream ----
        proj_unit(wqT, qT, 0, 0)()
        proj_unit(wkT, kT, 0, 0)()

        for qc in range(NQC):
            oT = work.tile([128, 2, QC], BF16, tag="oT", bufs=2)
            kmax = 4 * (qc + 1)
            for hp in range(2):
                last = (qc == NQC - 1 and hp == 1)
                drain_proj_through(("q", qc, hp))
                pts = {}
                po_box = [None]
                avs = [make_av(po_box, pts, kb, hp, kmax) for kb in range(kmax)]
                pending = [0]
                for kb in range(kmax):
                    if kb % 4 == 0:
                        drain_proj_through(("k", kb // 4, hp))
                    # S^T = k q^T, 2-head row-tiled pair, causally narrowed
                    cs = max(0, kb * 128 - qc * QC)
                    psS = ps_S.tile([128, 2, QC], F32, tag="S", bufs=2)
                    for hi in range(2):
                        nc.tensor.matmul(
                            psS[:, hi, cs:QC],
                            lhsT=kT[hi * 64:(hi + 1) * 64, hp,
                                    kb * 128:(kb + 1) * 128],
                            rhs=qT[hi * 64:(hi + 1) * 64, hp,
                                   qc * QC + cs:(qc + 1) * QC],
                            start=True,
                            stop=True,
                        )
                    pT = work.tile([128, 2, QC], BF16, tag="pT", bufs=30)
                    pts[kb] = (pT, cs)
                    pT_made[0] += 1
                    nc.scalar.activation(
                        pT[:, :, cs:QC],
                        psS[:, :, cs:QC],
                        mybir.ActivationFunctionType.Exp,
                        scale=SCALE,
                    )
                    if kb >= 4 * qc:  # diagonal band: zero the upper triangle
                        mask2 = bass.AP(
                            tensor=mask.tensor, offset=mask.offset,
                            ap=[mask.ap[0], [0, 2], mask.ap[1]],
                        )
                        nc.vector.tensor_mul(
                            pT[:, :, cs:cs + 128],
                            pT[:, :, cs:cs + 128],
                            mask2,
                        )
                    if last:
                        # drain the leftover queues, then pipeline this
                        # group's AV matmuls inline behind the exps
                        b = 3
                        while (attq or projq) and b > 0:
                            if attq:
                                w = attq[0][0]
                                pop_att()
                            else:
                                w = projq[0][0]
                                pop_proj()
                            b -= w
                        if not attq and not projq:
                            while pending[0] < kb:
                                avs[pending[0]]()
                                pending[0] += 1
                    else:
                        slot_i[0] += 1
                        at = min(3, (W_ATT * max(0, slot_i[0] - 4)) // 60
                                 - att_drained[0])
                        while attq and at > 0:
                            at -= attq[0][0]
                            pop_att()
                        pt = min(6, (W_PROJ * slot_i[0]) // 56 - proj_drained[0])
                        while projq and pt > 0:
                            pt -= projq[0][0]
                            pop_proj()
                        # backpressure: keep the pT pool from wrapping
                        while attq and pT_made[0] - pT_used[0] > 24:
                            pop_att()
                if last:
                    while attq or projq:
                        if attq:
                            pop_att()
                        else:
                            pop_proj()
                    while pending[0] < kmax:
                        avs[pending[0]]()
                        pending[0] += 1
                    make_epilogue(po_box, oT, hp)()
                    for u in make_out_proj(qc, oT):
                        u()
                else:
                    attq.extend((1, 4 * qc + 3, u) for u in avs)
                    attq.append((1, None, make_epilogue(po_box, oT, hp)))
                    if hp == 1:
                        attq.extend((2, None, u) for u in make_out_proj(qc, oT))
        while attq or projq:
            if attq:
                pop_att()
            else:
                pop_proj()

    for p in [small, work, p01, perm, const]:
        p.release()


_CACHE = {}


def _build():
    if "nc" in _CACHE:
        return _CACHE["nc"]
    nc = bacc.Bacc("TRN2", target_bir_lowering=False, debug=False, num_devices=8)
    xT_d = nc.dram_tensor("xT", [NQC * 8 * 128, QC], BF16, kind="ExternalInput").ap()
    wqT_d = nc.dram_tensor("wqT", [D, E], BF16, kind="ExternalInput").ap()
    wkT_d = nc.dram_tensor("wkT", [D, E], BF16, kind="ExternalInput").ap()
    wvT_d = nc.dram_tensor("wvT", [D, E], BF16, kind="ExternalInput").ap()
    woT_d = nc.dram_tensor("woT", [E, D], BF16, kind="ExternalInput").ap()
    yT_d = nc.dram_tensor("yT", [D, S], BF16, kind="ExternalOutput").ap()
    mask_d = nc.dram_tensor("maskc", [128, 128], BF16, kind="ExternalInput").ap()
    ones_d = nc.dram_tensor("onesc", [128, 64], F32, kind="ExternalInput").ap()
    with tile.TileContext(nc) as tc:
        _emit(tc, nc, xT_d, wqT_d, wkT_d, wvT_d, woT_d, yT_d, mask_d, ones_d)
    nc.compile()
    _CACHE["nc"] = nc
    return nc


_r = np.arange(128)
_MASK = np.where(_r[:, None] <= _r[None, :], 1.0, 0.0).astype(ml_dtypes.bfloat16)
_ONES = np.ones((128, 64), dtype=np.float32)

LAST_RESULT = None


def kernel(x, wq, wk, wv, wo):
    global LAST_RESULT
    nc = _build()
    bf = ml_dtypes.bfloat16
    x = np.asarray(x, dtype=np.float32)
    wq = np.asarray(wq, dtype=np.float32)
    wk = np.asarray(wk, dtype=np.float32)
    wv = np.asarray(wv, dtype=np.float32)
    wo = np.asarray(wo, dtype=np.float32)

    # pre-blocked layout: [sc, dc, p, t] row-major so each (sc, dc) chunk
    # is a contiguous 128KB slab for the DMA engines
    xT = [
        np.ascontiguousarray(
            x[b].T.reshape(8, 128, NQC, QC).transpose(2, 0, 1, 3)
        ).astype(bf).reshape(NQC * 8 * 128, QC)
        for b in range(B)
    ]
    wqT, wkT, wvT, woT = [], [], [], []
    for g in range(4):
        rows = slice(g * E, (g + 1) * E)
        wqT.append(np.ascontiguousarray(wq[rows].T).astype(bf))
        wkT.append(np.ascontiguousarray(wk[rows].T).astype(bf))
        wvT.append(np.ascontiguousarray(wv[rows].T).astype(bf))
        woT.append(np.ascontiguousarray(wo[:, rows].T).astype(bf))

    in_maps = []
    for c in range(8):
        b, g = c // 4, c % 4
        in_maps.append({
            "xT": xT[b],
            "wqT": wqT[g],
            "wkT": wkT[g],
            "wvT": wvT[g],
            "woT": woT[g],
            "maskc": _MASK,
            "onesc": _ONES,
        })

    res = bass_utils.run_bass_kernel_spmd(nc, in_maps, core_ids=list(range(8)))
    LAST_RESULT = res

    y = np.empty((B, S, D), dtype=np.float32)
    for b in range(B):
        acc = res.results[4 * b]["yT"].astype(np.float32)
        for g in range(1, 4):
            acc += res.results[4 * b + g]["yT"].astype(np.float32)
        y[b] = acc.T
    return y
